# revision 4
# baseline (speedup 1.0000x reference)
"""Trainium2 Bass kernel for nn_Attention_32263794328002.

Dense attention: x:[16,384,32,32], w_qkv:[1152,384], drop_mask:[16,6,1024,1024].
qkv = 1x1conv(x); per (b,h): attn = softmax(mask(qT k * scale)); out = attn @ v.

Strategy: pure data-parallel over batch (2 batches per core, 8 cores).
Per (b, h): compute S^T[m,n] = k^T q on the PE (contraction d=64, two heads
packed at PE row offsets 0/64), exp on ScalarE (no max subtraction needed:
|scale*S| <~ 1.2), multiply by the bf16 complement mask on VectorE (2x mode),
then out2[d+1, n] = [v;1]^T @ p^T accumulated over m-tiles on the PE. The
65th output row is the softmax denominator; the host does the final divide
and layout transpose.
"""

import sys

for _p in ("/opt/trn_rl_repo", "/opt/pypackages"):
    if _p not in sys.path:
        sys.path.append(_p)

import numpy as np
import ml_dtypes

import concourse.bass as bass  # noqa: F401
import concourse.bacc as bacc
import concourse.tile as tile
from concourse import mybir
from concourse.bass_utils import run_bass_kernel_spmd

BF16 = mybir.dt.bfloat16
F32 = mybir.dt.float32

B, C, H, W = 16, 384, 32, 32
HEADS = 6
D = C // HEADS          # 64
N = H * W               # 1024
NCORES = 8
BPC = B // NCORES       # batches per core = 2
HP = HEADS // 2         # head pairs = 3
MT = N // 128           # m tiles = 8
SCALE = float(C) ** -0.5


def build_nc():
    nc = bacc.Bacc(None, target_bir_lowering=False, debug=False)

    x_d = nc.dram_tensor("x", [BPC, C, N], BF16, kind="ExternalInput")
    wT_d = nc.dram_tensor("wT", [C, 3 * C], BF16, kind="ExternalInput")
    mc_d = nc.dram_tensor("maskc", [BPC, HEADS, N, N], BF16, kind="ExternalInput")
    out_d = nc.dram_tensor("out", [BPC, HEADS, D + 1, N], BF16, kind="ExternalOutput")

    CT = C // 128  # 3 contraction tiles for qkv

    with tile.TileContext(nc) as tc:
        with (
            tc.tile_pool(name="singles", bufs=1) as singles,
            tc.tile_pool(name="xpool", bufs=2) as xpool,
            tc.tile_pool(name="qkpool", bufs=2) as qkpool,
            tc.tile_pool(name="vpool", bufs=2) as vpool,
            tc.tile_pool(name="mcpool", bufs=4) as mcpool,
            tc.tile_pool(name="ppool", bufs=4) as ppool,
            tc.tile_pool(name="opool", bufs=4) as opool,
            tc.tile_pool(name="psS", bufs=2, space="PSUM") as psS,
            tc.tile_pool(name="psO", bufs=1, space="PSUM") as psO,
        ):
            wT_sb = singles.tile([128, CT, 3 * C], BF16)
            for ct in range(CT):
                nc.sync.dma_start(
                    out=wT_sb[:, ct, :], in_=wT_d[ct * 128 : (ct + 1) * 128, :]
                )

            for b in range(BPC):
                # ---- qkv ----
                x_sb = xpool.tile([128, CT, N], BF16)
                for ct in range(CT):
                    nc.sync.dma_start(
                        out=x_sb[:, ct, :], in_=x_d[b, ct * 128 : (ct + 1) * 128, :]
                    )

                # q,k: [o, n] layout; o-tiles 0..2 are q (c 0:384), 3..5 are k
                qk_sb = qkpool.tile([128, 6, N], BF16)
                for ot in range(6):
                    ps = psS.tile([128, N], F32)
                    for nh in range(2):
                        for ct in range(CT):
                            nc.tensor.matmul(
                                ps[:, nh * 512 : (nh + 1) * 512],
                                wT_sb[:, ct, ot * 128 : (ot + 1) * 128],
                                x_sb[:, ct, nh * 512 : (nh + 1) * 512],
                                start=(ct == 0),
                                stop=(ct == CT - 1),
                            )
                    nc.vector.tensor_copy(out=qk_sb[:, ot, :], in_=ps[:, :])

                # v^T: [spatial m, c_v] with a ones column per head -> [m, h, 65]
                vT_sb = vpool.tile([128, MT, HEADS, D + 1], BF16)
                nc.vector.memset(vT_sb[:, :, :, D : D + 1], 1.0)
                for nt in range(MT):
                    ps = psS.tile([128, N], F32)
                    for ct in range(CT):
                        nc.tensor.matmul(
                            ps[:, 0:C],
                            x_sb[:, ct, nt * 128 : (nt + 1) * 128],
                            wT_sb[:, ct, 2 * C : 3 * C],
                            start=(ct == 0),
                            stop=(ct == CT - 1),
                        )
                    nc.vector.tensor_copy(
                        out=vT_sb[:, nt, :, 0:D],
                        in_=ps[:, 0:C].rearrange("p (h d) -> p h d", h=HEADS),
                    )

                # ---- attention, one head-pair at a time ----
                for hp in range(HP):
                    po = [
                        psO.tile([D + 1, N], F32, name=f"po{j}", tag=f"po{j}")
                        for j in range(2)
                    ]
                    for mt in range(MT):
                        mc = mcpool.tile([128, 2, N], BF16)
                        for j in range(2):
                            nc.sync.dma_start(
                                out=mc[:, j, :],
                                in_=mc_d[b, 2 * hp + j, mt * 128 : (mt + 1) * 128, :],
                            )
                        for j in range(2):
                            r0 = 64 * j
                            ps = psS.tile([128, N], F32)
                            for nh in range(2):
                                nc.tensor.matmul(
                                    ps[:, nh * 512 : (nh + 1) * 512],
                                    qk_sb[r0 : r0 + 64, 3 + hp, mt * 128 : (mt + 1) * 128],
                                    qk_sb[r0 : r0 + 64, hp, nh * 512 : (nh + 1) * 512],
                                    start=True,
                                    stop=True,
                                )
                            pT = ppool.tile([128, N], BF16)
                            nc.scalar.activation(
                                out=pT[:, :],
                                in_=ps[:, :],
                                func=mybir.ActivationFunctionType.Exp,
                                scale=SCALE,
                            )
                            nc.vector.tensor_mul(pT[:, :], pT[:, :], mc[:, j, :])
                            for nh in range(2):
                                nc.tensor.matmul(
                                    po[j][:, nh * 512 : (nh + 1) * 512],
                                    vT_sb[:, mt, 2 * hp + j, :],
                                    pT[:, nh * 512 : (nh + 1) * 512],
                                    start=(mt == 0),
                                    stop=(mt == MT - 1),
                                    skip_group_check=True,
                                )
                    for j in range(2):
                        ob = opool.tile([D + 1, N], BF16)
                        nc.vector.tensor_copy(out=ob[:, :], in_=po[j][:, :])
                        nc.gpsimd.dma_start(out=out_d[b, 2 * hp + j], in_=ob[:, :])

    nc.compile()
    return nc


_NC_CACHE = None


def _get_nc():
    global _NC_CACHE
    if _NC_CACHE is None:
        _NC_CACHE = build_nc()
    return _NC_CACHE


def prepare_in_maps(x, w_qkv, drop_mask):
    bf16 = ml_dtypes.bfloat16
    x_b = np.ascontiguousarray(x.reshape(B, C, N)).astype(bf16)
    wT = np.ascontiguousarray(w_qkv.T).astype(bf16)
    # complement mask, transposed to [b, h, m_key, n_query]
    mc = np.ascontiguousarray(
        (~drop_mask.astype(bool)).transpose(0, 1, 3, 2)
    ).astype(bf16)
    in_maps = []
    for c in range(NCORES):
        sl = slice(c * BPC, (c + 1) * BPC)
        in_maps.append({"x": x_b[sl], "wT": wT, "maskc": mc[sl]})
    return in_maps


def postprocess(results):
    outs = []
    for c in range(NCORES):
        o = np.asarray(results[c]["out"]).astype(np.float32)  # [BPC, h, 65, n]
        num = o[:, :, :D, :]
        den = o[:, :, D : D + 1, :]
        outs.append((num / den).reshape(BPC, C, H, W))
    return np.concatenate(outs, axis=0)


def kernel(x, w_qkv, drop_mask):
    nc = _get_nc()
    in_maps = prepare_in_maps(np.asarray(x), np.asarray(w_qkv), np.asarray(drop_mask))
    res = run_bass_kernel_spmd(nc, in_maps, core_ids=list(range(NCORES)))
    return postprocess(res.results)


if __name__ == "__main__":
    rng = np.random.default_rng(0)
    x = rng.standard_normal((B, C, H, W), dtype=np.float32)
    w = rng.standard_normal((3 * C, C), dtype=np.float32) * 0.05
    m = rng.random((B, HEADS, N, N)) < 0.1
    out = kernel(x=x, w_qkv=w, drop_mask=m)
    print(out.shape, out.dtype)
